# revision 29
# baseline (speedup 1.0000x reference)
"""Trainium2 Bass kernel for nn_MultiHeadAttention_65352222376626.

Reference (B=8, S=1024, D=768, H=12):
    q = einsum('bsd,hde->bhse', x, Wq) + bq     (full-width per-head proj)
    k, v likewise
    scores = einsum('bhse,bhte->bhst', q, k) * sqrt(64)
    attn = softmax(scores, -1)
    o = einsum('bhst,bhte->bhse', attn, v)
    out = concat_heads(o) @ Wp + bp

Algebraic folding (softmax is invariant to per-row shifts; rows of attn
sum to 1):
    scores ~ x A_h x^T + 1 (x w_h)^T          A_h = Wq_h Wk_h^T,  w_h = Wk_h bq_h
    out    = sum_h attn_h (x C_h) + bp_eff    C_h = Wv_h Wp_h
    bp_eff = bp + sum_h bv_h Wp_h             (bq row-term and bk drop entirely)
This removes the separate q/k/v and output projections: per head only
  M = x A_h   [S,D]   and   scores = M x^T   [S,S]
  N = x C_h   [S,D]   and   out += attn N    [S,D]

Precision: the softmax is near-argmax (logit std ~222), so scores need
~2^-16 relative accuracy.  Each of the two scores-path matmuls runs as
3 passes: fp16(hi)*fp16(hi) at 1.0 cyc/row plus TWO fp8e4m3 DoubleRow
correction passes at 0.5 cyc/row (lo*hi and hi*lo), with power-of-2
scale frames chosen so every fp8 operand sits in e4m3's normal range:
  A frame 2^18:  A18=f16(A*2^18), Al8=e4(A*2^18-A18), A8=e4(A*2^6)
  x frame 2^0 :  x16=f16(x), x8=e4(x), xl12=e4((x-x16)*2^12)
  M psum at 2^18 -> M16=f16(psum*2^-8) [2^10], Ml8=e4(psum*2^-8-M16),
                    M8=e4(psum*2^-20) [2^-2]
  scores psum at 2^10; exp(psum*(8/1024) - 8/1024*rowmax)
Verified on hardware (probe_dr.py): HW == host simulation exactly;
end-to-end absmax rel err ~1.5e-3.  The column shift g = x w_h is
host-computed (G, scale 2^10) and broadcast-added into the scores psum
via a K=1 float32r ones-matmul.  attn/N/out path is plain fp16 (error
there is linear, not argmax-amplified).

Sharding: pure batch-parallel, B == n_cores == 8, one batch element per
core, folded weights replicated.  No collectives.

Schedule: per head, PE does scores(st=0..7) with softmax lagging on
DVE/ACT and P-transposes lagging one tile; M(h+1) fills the softmax
drain; then out(h) and N(h+1).  PSUM: big pool 2x2 banks (scores/N/out),
tp 2x1, M 2x1 -> 8 banks.
"""

import numpy as np
import ml_dtypes

B, S, D, H = 8, 1024, 768, 12
P = 128
SD = S // P   # 8 s-tiles
ED = D // P   # 6 feature tiles
EXP_SCALE = 8.0 / 1024.0   # sqrt(64) / (scores psum scale 2^10)

SCHEME = "dr"   # "dr" = fp16+fp8 DoubleRow corrections; "f16" = fp16 3-pass

_CACHE = {}

E4 = ml_dtypes.float8_e4m3


def _build_nc(scheme=SCHEME):
    import concourse.tile as tile
    from concourse import bacc, mybir

    f32 = mybir.dt.float32
    f16 = mybir.dt.float16
    f8 = mybir.dt.float8e4
    AF = mybir.ActivationFunctionType
    ALU = mybir.AluOpType
    DR = mybir.MatmulPerfMode.DoubleRow
    dr = scheme == "dr"

    nc = bacc.Bacc()

    # ---- DRAM I/O (per core) ----
    x16_d = nc.dram_tensor("x16T", [D, S], f16, kind="ExternalInput")
    if dr:
        x8_d = nc.dram_tensor("x8T", [D, S], f8, kind="ExternalInput")
        xl12_d = nc.dram_tensor("xl12T", [D, S], f8, kind="ExternalInput")
    else:
        xl16_d = nc.dram_tensor("xl16T", [D, S], f16, kind="ExternalInput")
    a18_d = nc.dram_tensor("A18", [H, D, D], f16, kind="ExternalInput")
    if dr:
        al8_d = nc.dram_tensor("Al8", [H, D, D], f8, kind="ExternalInput")
        a8_d = nc.dram_tensor("A8", [H, D, D], f8, kind="ExternalInput")
    else:
        al16_d = nc.dram_tensor("Al16", [H, D, D], f16, kind="ExternalInput")
    c16_d = nc.dram_tensor("C16", [H, D, D], f16, kind="ExternalInput")
    w18_d = nc.dram_tensor("W18", [H, 1, D], f16, kind="ExternalInput")
    out_d = nc.dram_tensor("out", [S, D], f32, kind="ExternalOutput")

    # partition-tiled DRAM views
    tv = lambda d: d.rearrange("(o p) s -> p o s", p=P)          # [128,ED,S]
    wv = lambda d: d.rearrange("h (o p) e -> h p o e", p=P)      # [H,128,ED,D]
    x16_t = tv(x16_d)
    if dr:
        x8_t, xl12_t = tv(x8_d), tv(xl12_d)
    else:
        xl16_t = tv(xl16_d)
    a18_t = wv(a18_d)
    if dr:
        al8_t, a8_t = wv(al8_d), wv(a8_d)
    else:
        al16_t = wv(al16_d)
    c16_t = wv(c16_d)
    out_t = out_d.rearrange("(o p) d -> p o d", p=P)             # [128,SD,D]

    with tile.TileContext(nc) as tc:
        with (
            tc.tile_pool(name="persist", bufs=1) as persist,
            tc.tile_pool(name="astream", bufs=2) as astream,
            tc.tile_pool(name="mtiles", bufs=2) as mtiles,
            tc.tile_pool(name="work", bufs=2) as work,
            tc.tile_pool(name="small", bufs=4) as small,
            tc.tile_pool(name="bigps", bufs=2, space="PSUM") as bigps,
            tc.tile_pool(name="mps", bufs=2, space="PSUM") as mps,
        ):
            # ---- persistent tiles (DMA order: x16+A(0) first so M(0)
            # can start; fp8 pieces and C(0) land during it) ----
            x16 = persist.tile([P, ED, S], f16)
            nc.sync.dma_start(x16[:], x16_t)
            ones16 = persist.tile([1, 512], f16)
            nc.vector.memset(ones16[:], 1.0)
            acc = persist.tile([P, SD, D], f32)
            recips = persist.tile([P, SD], f32)
            pT = persist.tile([P, SD, S], f16)
            nsb = persist.tile([P, SD, D], f16)

            def load_head(h):
                a18 = astream.tile([P, ED, D], f16, tag="a18")
                nc.sync.dma_start(a18[:], a18_t[h])
                w18 = astream.tile([1, D], f16, tag="w18")
                nc.sync.dma_start(w18[:], w18_d[h])
                if dr:
                    al8 = astream.tile([P, ED, D], f8, tag="al8")
                    nc.sync.dma_start(al8[:], al8_t[h])
                    a8 = astream.tile([P, ED, D], f8, tag="a8")
                    nc.sync.dma_start(a8[:], a8_t[h])
                    return (a18, w18, al8, a8)
                al16 = astream.tile([P, ED, D], f16, tag="al16")
                nc.sync.dma_start(al16[:], al16_t[h])
                return (a18, w18, al16)

            def load_c(h):
                c16 = astream.tile([P, ED, D], f16, tag="c16")
                nc.sync.dma_start(c16[:], c16_t[h])
                return c16

            def build_m(atiles):
                """M psum (scale 2^18) -> M16 (f16, 2^10), Ml8, M8 (f8)."""
                m16 = mtiles.tile([P, ED, S], f16, tag="m16")
                if dr:
                    ml8 = mtiles.tile([P, ED, S], f8, tag="ml8")
                    m8 = mtiles.tile([P, ED, S], f8, tag="m8")
                else:
                    ml16 = mtiles.tile([P, ED, S], f16, tag="ml16")
                for et in range(ED):
                    e_sl = slice(et * P, (et + 1) * P)
                    for sc in range(2):
                        s_sl = slice(sc * 512, (sc + 1) * 512)
                        ps = mps.tile([P, 512], f32, tag="m")
                        if dr:
                            a18, w18, al8, a8 = atiles
                            for dt_ in range(ED):
                                nc.tensor.matmul(
                                    ps[:], a18[:, dt_, e_sl], x16[:, dt_, s_sl],
                                    start=(dt_ == 0), stop=False)
                            # + 1 (x) w^T: column shift of scores, folded in
                            # at scale 2^18 so the hi/lo split carries it
                            nc.tensor.matmul(
                                ps[:], w18[:, e_sl], ones16[:],
                                start=False, stop=False)
                            for a in range(ED // 2):
                                d2 = slice(2 * a, 2 * a + 2)
                                nc.tensor.matmul(
                                    ps[:], al8[:, d2, e_sl], x8[:, d2, s_sl],
                                    start=False, stop=False, perf_mode=DR)
                            for a in range(ED // 2):
                                d2 = slice(2 * a, 2 * a + 2)
                                nc.tensor.matmul(
                                    ps[:], a8[:, d2, e_sl], xl12[:, d2, s_sl],
                                    start=False, stop=(a == ED // 2 - 1),
                                    perf_mode=DR)
                        else:
                            a18, w18, al16 = atiles
                            for dt_ in range(ED):
                                nc.tensor.matmul(
                                    ps[:], a18[:, dt_, e_sl], x16[:, dt_, s_sl],
                                    start=(dt_ == 0), stop=False)
                                nc.tensor.matmul(
                                    ps[:], al16[:, dt_, e_sl], x16[:, dt_, s_sl],
                                    start=False, stop=False)
                                nc.tensor.matmul(
                                    ps[:], a18[:, dt_, e_sl], xl16[:, dt_, s_sl],
                                    start=False, stop=False)
                            nc.tensor.matmul(
                                ps[:], w18[:, e_sl], ones16[:],
                                start=False, stop=True)
                        nc.scalar.activation(
                            m16[:, et, s_sl], ps[:], AF.Copy, scale=2.0 ** -8)
                        if dr:
                            nc.vector.scalar_tensor_tensor(
                                ml8[:, et, s_sl], ps[:], 2.0 ** -8,
                                m16[:, et, s_sl], ALU.mult, ALU.subtract)
                            nc.scalar.activation(
                                m8[:, et, s_sl], ps[:], AF.Copy, scale=2.0 ** -20)
                        else:
                            nc.vector.scalar_tensor_tensor(
                                ml16[:, et, s_sl], ps[:], 2.0 ** -8,
                                m16[:, et, s_sl], ALU.mult, ALU.subtract)
                if dr:
                    return (m16, ml8, m8)
                return (m16, ml16)

            def build_n(c16):
                """N = x C (fp16), layout [t-part, tt, d]."""
                n = nsb
                for tt in range(SD):
                    t_sl = slice(tt * P, (tt + 1) * P)
                    ps = bigps.tile([P, D], f32, tag="big")
                    for (d0, d1) in ((0, 512), (512, D)):
                        for et in range(ED):
                            nc.tensor.matmul(
                                ps[:, d0:d1], x16[:, et, t_sl],
                                c16[:, et, d0:d1],
                                start=(et == 0), stop=(et == ED - 1))
                    nc.scalar.activation(n[:, tt, :], ps[:], AF.Copy)
                return n

            def scores_tile(st, mt):
                """scores psum for s-tile st -> P (unnorm, f16) + recip."""
                s_sl = slice(st * P, (st + 1) * P)
                sc_ps = bigps.tile([P, S], f32, tag="big")
                for tch in range(2):
                    t_sl = slice(tch * 512, (tch + 1) * 512)
                    if dr:
                        m16, ml8, m8 = mt
                        for et in range(ED):
                            nc.tensor.matmul(
                                sc_ps[:, t_sl], m16[:, et, s_sl],
                                x16[:, et, t_sl], start=(et == 0), stop=False)
                        for a in range(ED // 2):
                            e2 = slice(2 * a, 2 * a + 2)
                            nc.tensor.matmul(
                                sc_ps[:, t_sl], ml8[:, e2, s_sl],
                                x8[:, e2, t_sl],
                                start=False, stop=False, perf_mode=DR)
                        for a in range(ED // 2):
                            e2 = slice(2 * a, 2 * a + 2)
                            nc.tensor.matmul(
                                sc_ps[:, t_sl], m8[:, e2, s_sl],
                                xl12[:, e2, t_sl],
                                start=False, stop=(a == ED // 2 - 1),
                                perf_mode=DR)
                    else:
                        m16, ml16 = mt
                        for et in range(ED):
                            nc.tensor.matmul(
                                sc_ps[:, t_sl], m16[:, et, s_sl],
                                x16[:, et, t_sl], start=(et == 0), stop=False)
                            nc.tensor.matmul(
                                sc_ps[:, t_sl], ml16[:, et, s_sl],
                                x16[:, et, t_sl], start=False, stop=False)
                            nc.tensor.matmul(
                                sc_ps[:, t_sl], m16[:, et, s_sl],
                                xl16[:, et, t_sl], start=False,
                                stop=(et == ED - 1))
                negmax = small.tile([P, 1], f32, tag="negmax")
                nc.vector.tensor_reduce(
                    negmax[:], sc_ps[:], axis=mybir.AxisListType.X,
                    op=mybir.AluOpType.max, negate=True)
                bias8 = small.tile([P, 1], f32, tag="bias8")
                nc.vector.tensor_scalar_mul(bias8[:], negmax[:], EXP_SCALE)
                ptile = work.tile([P, S], f16, tag="p")
                sumexp = small.tile([P, 1], f32, tag="sumexp")
                nc.scalar.activation(
                    ptile[:], sc_ps[:], AF.Exp,
                    bias=bias8[:], scale=EXP_SCALE, accum_out=sumexp[:])
                nc.vector.reciprocal(recips[:, st:st + 1], sumexp[:])
                return ptile

            def transpose_p(st, ptile):
                # P^T via the DMA xbar transpose engine (PE stays free)
                s_sl = slice(st * P, (st + 1) * P)
                for tt in range(SD):
                    t_sl = slice(tt * P, (tt + 1) * P)
                    nc.sync.dma_start_transpose(
                        pT[:, tt, s_sl], ptile[:, t_sl])

            def out_tile(st, n):
                s_sl = slice(st * P, (st + 1) * P)
                ps = bigps.tile([P, D], f32, tag="big")
                for (d0, d1) in ((0, 512), (512, D)):
                    for tt in range(SD):
                        nc.tensor.matmul(
                            ps[:, d0:d1], pT[:, tt, s_sl], n[:, tt, d0:d1],
                            start=(tt == 0), stop=(tt == SD - 1))
                nc.vector.scalar_tensor_tensor(
                    acc[:, st, :], ps[:], recips[:, st:st + 1],
                    acc[:, st, :], ALU.mult, ALU.add)

            # ---- prologue ----
            nc.vector.memset(acc[:], 0.0)
            atiles = load_head(0)
            if dr:
                x8 = persist.tile([P, ED, S], f8)
                nc.sync.dma_start(x8[:], x8_t)
                xl12 = persist.tile([P, ED, S], f8)
                nc.sync.dma_start(xl12[:], xl12_t)
            else:
                xl16 = persist.tile([P, ED, S], f16)
                nc.sync.dma_start(xl16[:], xl16_t)
            c16 = load_c(0)
            mt = build_m(atiles)
            n = build_n(c16)

            # ---- head loop (software-pipelined) ----
            for h in range(H):
                if h + 1 < H:
                    atiles_n = load_head(h + 1)
                    c16_n = load_c(h + 1)
                ptiles = {}
                for st in range(SD):
                    ptiles[st] = scores_tile(st, mt)
                    if st >= 1:
                        transpose_p(st - 1, ptiles[st - 1])
                        del ptiles[st - 1]
                if h + 1 < H:
                    mt_next = build_m(atiles_n)
                transpose_p(SD - 1, ptiles[SD - 1])
                for st in range(SD):
                    out_tile(st, n)
                    if h == H - 1:
                        nc.sync.dma_start(out_t[:, st, :], acc[:, st, :])
                if h + 1 < H:
                    mt = mt_next
                    n = build_n(c16_n)

    nc.compile()
    return nc


def _get_nc():
    if "nc" not in _CACHE:
        _CACHE["nc"] = _build_nc()
    return _CACHE["nc"]


def _prepare(x, Wq, bq, Wk, bk, Wv, bv, Wp, bp):
    x = np.asarray(x, dtype=np.float32)
    Wq = np.asarray(Wq, dtype=np.float32)
    Wk = np.asarray(Wk, dtype=np.float32)
    Wv = np.asarray(Wv, dtype=np.float32)
    Wp = np.asarray(Wp, dtype=np.float32).reshape(H, D, D)
    bq = np.asarray(bq, dtype=np.float32)
    bv = np.asarray(bv, dtype=np.float32)
    bp = np.asarray(bp, dtype=np.float32)

    # folded weights
    A = np.matmul(Wq, np.transpose(Wk, (0, 2, 1)))          # [H,D,D] x A x^T
    C = np.matmul(Wv, Wp)                                   # [H,D,D]
    w = np.einsum('hde,he->hd', Wk, bq)                     # [H,D] col shift
    bp_eff = (bp.astype(np.float64)
              + np.einsum('hd,hde->e', bv.astype(np.float64),
                          Wp.astype(np.float64))).astype(np.float32)

    A18f = A * np.float32(2.0 ** 18)
    A18 = np.clip(A18f, -65504, 65504).astype(np.float16)
    Alr = A18f - A18.astype(np.float32)
    C16 = C.astype(np.float16)

    shared = {"A18": A18, "C16": C16}
    if SCHEME == "dr":
        shared["Al8"] = np.clip(Alr, -240, 240).astype(E4)
        shared["A8"] = np.clip(A * np.float32(2.0 ** 6), -240, 240).astype(E4)
    else:
        shared["Al16"] = Alr.astype(np.float16)

    # column shift w at the M-stage 2^18 scale frame
    shared["W18"] = np.clip(w * np.float32(2.0 ** 18),
                            -65504, 65504).astype(np.float16)[:, None, :]

    in_maps = []
    for b in range(B):
        xT = np.ascontiguousarray(x[b].T)
        x16 = xT.astype(np.float16)
        xl = xT - x16.astype(np.float32)
        m = {"x16T": x16, **shared}
        if SCHEME == "dr":
            m["x8T"] = np.clip(xT, -240, 240).astype(E4)
            m["xl12T"] = np.clip(xl * np.float32(2.0 ** 12), -240, 240).astype(E4)
        else:
            m["xl16T"] = xl.astype(np.float16)
        in_maps.append(m)
    return in_maps, bp_eff


def kernel(x, Wq, bq, Wk, bk, Wv, bv, Wp, bp):
    from concourse.bass_utils import run_bass_kernel_spmd

    in_maps, bp_eff = _prepare(x, Wq, bq, Wk, bk, Wv, bv, Wp, bp)
    nc = _get_nc()
    res = run_bass_kernel_spmd(nc, in_maps, list(range(B)))
    out = np.stack([res.results[b]["out"] for b in range(B)], axis=0)
    out = out + bp_eff[None, None, :]
    return out.astype(np.float32)


# revision 36
# speedup vs baseline: 1.0055x; 1.0055x over previous
"""Trainium2 Bass kernel for nn_MultiHeadAttention_65352222376626.

Reference (B=8, S=1024, D=768, H=12):
    q = einsum('bsd,hde->bhse', x, Wq) + bq     (full-width per-head proj)
    k, v likewise
    scores = einsum('bhse,bhte->bhst', q, k) * sqrt(64)
    attn = softmax(scores, -1)
    o = einsum('bhst,bhte->bhse', attn, v)
    out = concat_heads(o) @ Wp + bp

Algebraic folding (softmax is invariant to per-row shifts; rows of attn
sum to 1):
    scores ~ x A_h x^T + 1 (x w_h)^T          A_h = Wq_h Wk_h^T,  w_h = Wk_h bq_h
    out    = sum_h attn_h (x C_h) + bp_eff    C_h = Wv_h Wp_h
    bp_eff = bp + sum_h bv_h Wp_h             (bq row-term and bk drop entirely)
This removes the separate q/k/v and output projections: per head only
  M = x A_h   [S,D]   and   scores = M x^T   [S,S]
  N = x C_h   [S,D]   and   out += attn N    [S,D]

Precision: the softmax is near-argmax (logit std ~222), so scores need
~2^-16 relative accuracy.  Each of the two scores-path matmuls runs as
3 passes: fp16(hi)*fp16(hi) at 1.0 cyc/row plus TWO fp8e4m3 DoubleRow
correction passes at 0.5 cyc/row (lo*hi and hi*lo), with power-of-2
scale frames chosen so every fp8 operand sits in e4m3's normal range:
  A frame 2^18:  A18=f16(A*2^18), Al8=e4(A*2^18-A18), A8=e4(A*2^6)
  x frame 2^0 :  x16=f16(x), x8=e4(x), xl12=e4((x-x16)*2^12)
  M psum at 2^18 -> M16=f16(psum*2^-8) [2^10], Ml8=e4(psum*2^-8-M16),
                    M8=e4(psum*2^-20) [2^-2]
  scores psum at 2^10; exp(psum*(8/1024) - 8/1024*rowmax)
Verified on hardware (probe_dr.py): HW == host simulation exactly;
end-to-end absmax rel err ~1.5e-3.  The column shift g = x w_h is
host-computed (G, scale 2^10) and broadcast-added into the scores psum
via a K=1 float32r ones-matmul.  attn/N/out path is plain fp16 (error
there is linear, not argmax-amplified).

Sharding: pure batch-parallel, B == n_cores == 8, one batch element per
core, folded weights replicated.  No collectives.

Schedule: per head, PE does scores(st=0..7) with softmax lagging on
DVE/ACT and P-transposes lagging one tile; M(h+1) fills the softmax
drain; then out(h) and N(h+1).  PSUM: big pool 2x2 banks (scores/N/out),
tp 2x1, M 2x1 -> 8 banks.
"""

import numpy as np
import ml_dtypes

B, S, D, H = 8, 1024, 768, 12
P = 128
SD = S // P   # 8 s-tiles
ED = D // P   # 6 feature tiles
EXP_SCALE = 8.0 / 1024.0   # sqrt(64) / (scores psum scale 2^10)

SCHEME = "dr"   # "dr" = fp16+fp8 DoubleRow corrections; "f16" = fp16 3-pass

_CACHE = {}

E4 = ml_dtypes.float8_e4m3


def _build_nc(scheme=SCHEME):
    import concourse.tile as tile
    from concourse import bacc, mybir
    from concourse.masks import make_identity

    f32 = mybir.dt.float32
    f16 = mybir.dt.float16
    f8 = mybir.dt.float8e4
    AF = mybir.ActivationFunctionType
    ALU = mybir.AluOpType
    DR = mybir.MatmulPerfMode.DoubleRow
    dr = scheme == "dr"

    nc = bacc.Bacc()

    # ---- DRAM I/O (per core) ----
    x16_d = nc.dram_tensor("x16T", [D, S], f16, kind="ExternalInput")
    if dr:
        x8_d = nc.dram_tensor("x8T", [D, S], f8, kind="ExternalInput")
        xl12_d = nc.dram_tensor("xl12T", [D, S], f8, kind="ExternalInput")
    else:
        xl16_d = nc.dram_tensor("xl16T", [D, S], f16, kind="ExternalInput")
    a18_d = nc.dram_tensor("A18", [H, D, D], f16, kind="ExternalInput")
    if dr:
        al8_d = nc.dram_tensor("Al8", [H, D, D], f8, kind="ExternalInput")
        a8_d = nc.dram_tensor("A8", [H, D, D], f8, kind="ExternalInput")
    else:
        al16_d = nc.dram_tensor("Al16", [H, D, D], f16, kind="ExternalInput")
    c16_d = nc.dram_tensor("C16", [H, D, D], f16, kind="ExternalInput")
    w18_d = nc.dram_tensor("W18", [H, 1, D], f16, kind="ExternalInput")
    out_d = nc.dram_tensor("out", [S, D], f32, kind="ExternalOutput")

    # partition-tiled DRAM views
    tv = lambda d: d.rearrange("(o p) s -> p o s", p=P)          # [128,ED,S]
    wv = lambda d: d.rearrange("h (o p) e -> h p o e", p=P)      # [H,128,ED,D]
    x16_t = tv(x16_d)
    if dr:
        x8_t, xl12_t = tv(x8_d), tv(xl12_d)
    else:
        xl16_t = tv(xl16_d)
    a18_t = wv(a18_d)
    if dr:
        al8_t, a8_t = wv(al8_d), wv(a8_d)
    else:
        al16_t = wv(al16_d)
    c16_t = wv(c16_d)
    out_t = out_d.rearrange("(o p) d -> p o d", p=P)             # [128,SD,D]

    with tile.TileContext(nc) as tc:
        with (
            tc.tile_pool(name="persist", bufs=1) as persist,
            tc.tile_pool(name="astream", bufs=2) as astream,
            tc.tile_pool(name="mtiles", bufs=2) as mtiles,
            tc.tile_pool(name="work", bufs=2) as work,
            tc.tile_pool(name="small", bufs=4) as small,
            tc.tile_pool(name="bigps", bufs=2, space="PSUM") as bigps,
            tc.tile_pool(name="mps", bufs=2, space="PSUM") as mps,
            tc.tile_pool(name="tpps", bufs=1, space="PSUM") as tpps,
        ):
            # ---- persistent tiles (DMA order: x16+A(0) first so M(0)
            # can start; fp8 pieces and C(0) land during it) ----
            x16 = persist.tile([P, ED, S], f16)
            nc.sync.dma_start(x16[:], x16_t)
            ones16 = persist.tile([1, 512], f16)
            nc.vector.memset(ones16[:], 1.0)
            ident = persist.tile([P, P], f16)
            make_identity(nc, ident)
            acc = persist.tile([P, SD, D], f32)
            recips = persist.tile([P, SD], f32)
            pT = persist.tile([P, SD, S], f16)
            nsb = persist.tile([P, SD, D], f16)

            def load_head(h):
                a18 = astream.tile([P, ED, D], f16, tag="a18")
                nc.sync.dma_start(a18[:], a18_t[h])
                w18 = astream.tile([1, D], f16, tag="w18")
                nc.sync.dma_start(w18[:], w18_d[h])
                if dr:
                    al8 = astream.tile([P, ED, D], f8, tag="al8")
                    nc.sync.dma_start(al8[:], al8_t[h])
                    a8 = astream.tile([P, ED, D], f8, tag="a8")
                    nc.sync.dma_start(a8[:], a8_t[h])
                    return (a18, w18, al8, a8)
                al16 = astream.tile([P, ED, D], f16, tag="al16")
                nc.sync.dma_start(al16[:], al16_t[h])
                return (a18, w18, al16)

            def load_c(h):
                c16 = astream.tile([P, ED, D], f16, tag="c16")
                nc.sync.dma_start(c16[:], c16_t[h])
                return c16

            def build_m(atiles):
                """M psum (scale 2^18) -> M16 (f16, 2^10), Ml8, M8 (f8)."""
                m16 = mtiles.tile([P, ED, S], f16, tag="m16")
                if dr:
                    ml8 = mtiles.tile([P, ED, S], f8, tag="ml8")
                    m8 = mtiles.tile([P, ED, S], f8, tag="m8")
                else:
                    ml16 = mtiles.tile([P, ED, S], f16, tag="ml16")
                for et in range(ED):
                    e_sl = slice(et * P, (et + 1) * P)
                    for sc in range(2):
                        s_sl = slice(sc * 512, (sc + 1) * 512)
                        ps = mps.tile([P, 512], f32, tag="m")
                        if dr:
                            a18, w18, al8, a8 = atiles
                            for dt_ in range(ED):
                                nc.tensor.matmul(
                                    ps[:], a18[:, dt_, e_sl], x16[:, dt_, s_sl],
                                    start=(dt_ == 0), stop=False)
                            # + 1 (x) w^T: column shift of scores, folded in
                            # at scale 2^18 so the hi/lo split carries it
                            nc.tensor.matmul(
                                ps[:], w18[:, e_sl], ones16[:],
                                start=False, stop=False)
                            for a in range(ED // 2):
                                d2 = slice(2 * a, 2 * a + 2)
                                nc.tensor.matmul(
                                    ps[:], al8[:, d2, e_sl], x8[:, d2, s_sl],
                                    start=False, stop=False, perf_mode=DR)
                            for a in range(ED // 2):
                                d2 = slice(2 * a, 2 * a + 2)
                                nc.tensor.matmul(
                                    ps[:], a8[:, d2, e_sl], xl12[:, d2, s_sl],
                                    start=False, stop=(a == ED // 2 - 1),
                                    perf_mode=DR)
                        else:
                            a18, w18, al16 = atiles
                            for dt_ in range(ED):
                                nc.tensor.matmul(
                                    ps[:], a18[:, dt_, e_sl], x16[:, dt_, s_sl],
                                    start=(dt_ == 0), stop=False)
                                nc.tensor.matmul(
                                    ps[:], al16[:, dt_, e_sl], x16[:, dt_, s_sl],
                                    start=False, stop=False)
                                nc.tensor.matmul(
                                    ps[:], a18[:, dt_, e_sl], xl16[:, dt_, s_sl],
                                    start=False, stop=False)
                            nc.tensor.matmul(
                                ps[:], w18[:, e_sl], ones16[:],
                                start=False, stop=True)
                        nc.scalar.activation(
                            m16[:, et, s_sl], ps[:], AF.Copy, scale=2.0 ** -8)
                        if dr:
                            nc.vector.scalar_tensor_tensor(
                                ml8[:, et, s_sl], ps[:], 2.0 ** -8,
                                m16[:, et, s_sl], ALU.mult, ALU.subtract)
                            nc.scalar.activation(
                                m8[:, et, s_sl], ps[:], AF.Copy, scale=2.0 ** -20)
                        else:
                            nc.vector.scalar_tensor_tensor(
                                ml16[:, et, s_sl], ps[:], 2.0 ** -8,
                                m16[:, et, s_sl], ALU.mult, ALU.subtract)
                if dr:
                    return (m16, ml8, m8)
                return (m16, ml16)

            def build_n(c16):
                """N = x C (fp16), layout [t-part, tt, d]."""
                n = nsb
                for tt in range(SD):
                    t_sl = slice(tt * P, (tt + 1) * P)
                    ps = bigps.tile([P, D], f32, tag="big")
                    for (d0, d1) in ((0, 512), (512, D)):
                        for et in range(ED):
                            nc.tensor.matmul(
                                ps[:, d0:d1], x16[:, et, t_sl],
                                c16[:, et, d0:d1],
                                start=(et == 0), stop=(et == ED - 1))
                    nc.scalar.activation(n[:, tt, :], ps[:], AF.Copy)
                return n

            def scores_tile(st, mt):
                """scores psum for s-tile st -> P (unnorm, f16) + recip."""
                s_sl = slice(st * P, (st + 1) * P)
                sc_ps = bigps.tile([P, S], f32, tag="big")
                for tch in range(2):
                    t_sl = slice(tch * 512, (tch + 1) * 512)
                    if dr:
                        m16, ml8, m8 = mt
                        for et in range(ED):
                            nc.tensor.matmul(
                                sc_ps[:, t_sl], m16[:, et, s_sl],
                                x16[:, et, t_sl], start=(et == 0), stop=False)
                        for a in range(ED // 2):
                            e2 = slice(2 * a, 2 * a + 2)
                            nc.tensor.matmul(
                                sc_ps[:, t_sl], ml8[:, e2, s_sl],
                                x8[:, e2, t_sl],
                                start=False, stop=False, perf_mode=DR)
                        for a in range(ED // 2):
                            e2 = slice(2 * a, 2 * a + 2)
                            nc.tensor.matmul(
                                sc_ps[:, t_sl], m8[:, e2, s_sl],
                                xl12[:, e2, t_sl],
                                start=False, stop=(a == ED // 2 - 1),
                                perf_mode=DR)
                    else:
                        m16, ml16 = mt
                        for et in range(ED):
                            nc.tensor.matmul(
                                sc_ps[:, t_sl], m16[:, et, s_sl],
                                x16[:, et, t_sl], start=(et == 0), stop=False)
                            nc.tensor.matmul(
                                sc_ps[:, t_sl], ml16[:, et, s_sl],
                                x16[:, et, t_sl], start=False, stop=False)
                            nc.tensor.matmul(
                                sc_ps[:, t_sl], m16[:, et, s_sl],
                                xl16[:, et, t_sl], start=False,
                                stop=(et == ED - 1))
                negmax = small.tile([P, 1], f32, tag="negmax")
                nc.vector.tensor_reduce(
                    negmax[:], sc_ps[:], axis=mybir.AxisListType.X,
                    op=mybir.AluOpType.max, negate=True)
                bias8 = small.tile([P, 1], f32, tag="bias8")
                nc.vector.tensor_scalar_mul(bias8[:], negmax[:], EXP_SCALE)
                ptile = work.tile([P, S], f16, tag="p")
                sumexp = small.tile([P, 1], f32, tag="sumexp")
                nc.scalar.activation(
                    ptile[:], sc_ps[:], AF.Exp,
                    bias=bias8[:], scale=EXP_SCALE, accum_out=sumexp[:])
                nc.vector.reciprocal(recips[:, st:st + 1], sumexp[:])
                return ptile

            def transpose_p(st, ptile):
                # P^T via the DMA xbar transpose engine (PE stays free),
                # alternating the two HWDGE dispatchers (SP / Activation)
                s_sl = slice(st * P, (st + 1) * P)
                for tt in range(SD):
                    t_sl = slice(tt * P, (tt + 1) * P)
                    nc.sync.dma_start_transpose(
                        pT[:, tt, s_sl], ptile[:, t_sl])

            def transpose_p_pe(st, ptile):
                # last s-tile sits on the critical path into out(); PE
                # transposes avoid waiting on the DMA xbar backlog
                s_sl = slice(st * P, (st + 1) * P)
                tp_ps = tpps.tile([P, SD, P], f16, tag="tp")
                for tt in range(SD):
                    t_sl = slice(tt * P, (tt + 1) * P)
                    nc.tensor.transpose(
                        tp_ps[:, tt, :], ptile[:, t_sl], ident[:])
                nc.vector.tensor_copy(pT[:, :, s_sl], tp_ps[:])

            def out_tile(st, n):
                s_sl = slice(st * P, (st + 1) * P)
                ps = bigps.tile([P, D], f32, tag="big")
                for (d0, d1) in ((0, 512), (512, D)):
                    for tt in range(SD):
                        nc.tensor.matmul(
                            ps[:, d0:d1], pT[:, tt, s_sl], n[:, tt, d0:d1],
                            start=(tt == 0), stop=(tt == SD - 1))
                nc.vector.scalar_tensor_tensor(
                    acc[:, st, :], ps[:], recips[:, st:st + 1],
                    acc[:, st, :], ALU.mult, ALU.add)

            # ---- prologue ----
            nc.vector.memset(acc[:], 0.0)
            atiles = load_head(0)
            if dr:
                x8 = persist.tile([P, ED, S], f8)
                nc.sync.dma_start(x8[:], x8_t)
                xl12 = persist.tile([P, ED, S], f8)
                nc.sync.dma_start(xl12[:], xl12_t)
            else:
                xl16 = persist.tile([P, ED, S], f16)
                nc.sync.dma_start(xl16[:], xl16_t)
            c16 = load_c(0)
            mt = build_m(atiles)
            n = build_n(c16)

            # ---- head loop (software-pipelined) ----
            for h in range(H):
                if h + 1 < H:
                    atiles_n = load_head(h + 1)
                    c16_n = load_c(h + 1)
                ptiles = {}
                for st in range(SD):
                    ptiles[st] = scores_tile(st, mt)
                    if st >= 1:
                        transpose_p(st - 1, ptiles[st - 1])
                        del ptiles[st - 1]
                if h + 1 < H:
                    mt_next = build_m(atiles_n)
                transpose_p_pe(SD - 1, ptiles[SD - 1])
                for st in range(SD):
                    out_tile(st, n)
                    if h == H - 1:
                        nc.sync.dma_start(out_t[:, st, :], acc[:, st, :])
                if h + 1 < H:
                    mt = mt_next
                    n = build_n(c16_n)

    nc.compile()
    return nc


def _get_nc():
    if "nc" not in _CACHE:
        _CACHE["nc"] = _build_nc()
    return _CACHE["nc"]


def _prepare(x, Wq, bq, Wk, bk, Wv, bv, Wp, bp):
    x = np.asarray(x, dtype=np.float32)
    Wq = np.asarray(Wq, dtype=np.float32)
    Wk = np.asarray(Wk, dtype=np.float32)
    Wv = np.asarray(Wv, dtype=np.float32)
    Wp = np.asarray(Wp, dtype=np.float32).reshape(H, D, D)
    bq = np.asarray(bq, dtype=np.float32)
    bv = np.asarray(bv, dtype=np.float32)
    bp = np.asarray(bp, dtype=np.float32)

    # folded weights
    A = np.matmul(Wq, np.transpose(Wk, (0, 2, 1)))          # [H,D,D] x A x^T
    C = np.matmul(Wv, Wp)                                   # [H,D,D]
    w = np.einsum('hde,he->hd', Wk, bq)                     # [H,D] col shift
    bp_eff = (bp.astype(np.float64)
              + np.einsum('hd,hde->e', bv.astype(np.float64),
                          Wp.astype(np.float64))).astype(np.float32)

    A18f = A * np.float32(2.0 ** 18)
    A18 = np.clip(A18f, -65504, 65504).astype(np.float16)
    Alr = A18f - A18.astype(np.float32)
    C16 = C.astype(np.float16)

    shared = {"A18": A18, "C16": C16}
    if SCHEME == "dr":
        shared["Al8"] = np.clip(Alr, -240, 240).astype(E4)
        shared["A8"] = np.clip(A * np.float32(2.0 ** 6), -240, 240).astype(E4)
    else:
        shared["Al16"] = Alr.astype(np.float16)

    # column shift w at the M-stage 2^18 scale frame
    shared["W18"] = np.clip(w * np.float32(2.0 ** 18),
                            -65504, 65504).astype(np.float16)[:, None, :]

    in_maps = []
    for b in range(B):
        xT = np.ascontiguousarray(x[b].T)
        x16 = xT.astype(np.float16)
        xl = xT - x16.astype(np.float32)
        m = {"x16T": x16, **shared}
        if SCHEME == "dr":
            m["x8T"] = np.clip(xT, -240, 240).astype(E4)
            m["xl12T"] = np.clip(xl * np.float32(2.0 ** 12), -240, 240).astype(E4)
        else:
            m["xl16T"] = xl.astype(np.float16)
        in_maps.append(m)
    return in_maps, bp_eff


def kernel(x, Wq, bq, Wk, bk, Wv, bv, Wp, bp):
    from concourse.bass_utils import run_bass_kernel_spmd

    in_maps, bp_eff = _prepare(x, Wq, bq, Wk, bk, Wv, bv, Wp, bp)
    nc = _get_nc()
    res = run_bass_kernel_spmd(nc, in_maps, list(range(B)))
    out = np.stack([res.results[b]["out"] for b in range(B)], axis=0)
    out = out + bp_eff[None, None, :]
    return out.astype(np.float32)


# revision 39
# speedup vs baseline: 1.2149x; 1.2083x over previous
"""Trainium2 Bass kernel for nn_MultiHeadAttention_65352222376626.

Reference (B=8, S=1024, D=768, H=12):
    q = einsum('bsd,hde->bhse', x, Wq) + bq     (full-width per-head proj)
    k, v likewise
    scores = einsum('bhse,bhte->bhst', q, k) * sqrt(64)
    attn = softmax(scores, -1)
    o = einsum('bhst,bhte->bhse', attn, v)
    out = concat_heads(o) @ Wp + bp

Algebraic folding (softmax is invariant to per-row shifts; rows of attn
sum to 1):
    scores ~ x A_h x^T + 1 (x w_h)^T          A_h = Wq_h Wk_h^T,  w_h = Wk_h bq_h
    out    = sum_h attn_h (x C_h) + bp_eff    C_h = Wv_h Wp_h
    bp_eff = bp + sum_h bv_h Wp_h             (bq row-term and bk drop entirely)
This removes the separate q/k/v and output projections: per head only
  M = x A_h   [S,D]   and   scores = M x^T   [S,S]
  N = x C_h   [S,D]   and   out += attn N    [S,D]

Precision: the softmax is near-argmax (logit std ~222), so scores need
~2^-16 relative accuracy.  Each of the two scores-path matmuls runs as
3 passes: fp16(hi)*fp16(hi) at 1.0 cyc/row plus TWO fp8e4m3 DoubleRow
correction passes at 0.5 cyc/row (lo*hi and hi*lo), with power-of-2
scale frames chosen so every fp8 operand sits in e4m3's normal range:
  A frame 2^18:  A18=f16(A*2^18), Al8=e4(A*2^18-A18), A8=e4(A*2^6)
  x frame 2^0 :  x16=f16(x), x8=e4(x), xl12=e4((x-x16)*2^12)
  M psum at 2^18 -> M16=f16(psum*2^-8) [2^10], Ml8=e4(psum*2^-8-M16),
                    M8=e4(psum*2^-20) [2^-2]
  scores psum at 2^10; exp(psum*(8/1024) - 8/1024*rowmax)
Verified on hardware (probe_dr.py): HW == host simulation exactly;
end-to-end absmax rel err ~1.5e-3.  The column shift g = x w_h is
host-computed (G, scale 2^10) and broadcast-added into the scores psum
via a K=1 float32r ones-matmul.  attn/N/out path is plain fp16 (error
there is linear, not argmax-amplified).

Sharding: pure batch-parallel, B == n_cores == 8, one batch element per
core, folded weights replicated.  No collectives.

Schedule: per head, PE does scores(st=0..7) with softmax lagging on
DVE/ACT and P-transposes lagging one tile; M(h+1) fills the softmax
drain; then out(h) and N(h+1).  PSUM: big pool 2x2 banks (scores/N/out),
tp 2x1, M 2x1 -> 8 banks.
"""

import numpy as np
import ml_dtypes

B, S, D, H = 8, 1024, 768, 12
P = 128
SD = S // P   # 8 s-tiles
ED = D // P   # 6 feature tiles
EXP_SCALE = 8.0 / 1024.0   # sqrt(64) / (scores psum scale 2^10)

SCHEME = "dr"   # "dr" = fp16+fp8 DoubleRow corrections; "f16" = fp16 3-pass

_CACHE = {}

E4 = ml_dtypes.float8_e4m3


def _build_nc(scheme=SCHEME):
    import concourse.tile as tile
    from concourse import bacc, mybir
    from concourse.masks import make_identity

    f32 = mybir.dt.float32
    f16 = mybir.dt.float16
    f8 = mybir.dt.float8e4
    AF = mybir.ActivationFunctionType
    ALU = mybir.AluOpType
    DR = mybir.MatmulPerfMode.DoubleRow
    dr = scheme == "dr"

    nc = bacc.Bacc()

    # ---- DRAM I/O (per core) ----
    x16_d = nc.dram_tensor("x16T", [D, S], f16, kind="ExternalInput")
    if dr:
        x8_d = nc.dram_tensor("x8T", [D, S], f8, kind="ExternalInput")
        xl12_d = nc.dram_tensor("xl12T", [D, S], f8, kind="ExternalInput")
    else:
        xl16_d = nc.dram_tensor("xl16T", [D, S], f16, kind="ExternalInput")
    a18_d = nc.dram_tensor("A18", [H, D, D], f16, kind="ExternalInput")
    if dr:
        al8_d = nc.dram_tensor("Al8", [H, D, D], f8, kind="ExternalInput")
        a8_d = nc.dram_tensor("A8", [H, D, D], f8, kind="ExternalInput")
    else:
        al16_d = nc.dram_tensor("Al16", [H, D, D], f16, kind="ExternalInput")
    c16_d = nc.dram_tensor("C16", [H, D, D], f16, kind="ExternalInput")
    w18_d = nc.dram_tensor("W18", [H, 1, D], f16, kind="ExternalInput")
    out_d = nc.dram_tensor("out", [S, D], f32, kind="ExternalOutput")

    # partition-tiled DRAM views
    tv = lambda d: d.rearrange("(o p) s -> p o s", p=P)          # [128,ED,S]
    wv = lambda d: d.rearrange("h (o p) e -> h p o e", p=P)      # [H,128,ED,D]
    x16_t = tv(x16_d)
    if dr:
        x8_t, xl12_t = tv(x8_d), tv(xl12_d)
    else:
        xl16_t = tv(xl16_d)
    a18_t = wv(a18_d)
    if dr:
        al8_t, a8_t = wv(al8_d), wv(a8_d)
    else:
        al16_t = wv(al16_d)
    c16_t = wv(c16_d)
    out_t = out_d.rearrange("(o p) d -> p o d", p=P)             # [128,SD,D]

    with tile.TileContext(nc) as tc:
        with (
            tc.tile_pool(name="persist", bufs=1) as persist,
            tc.tile_pool(name="astream", bufs=2) as astream,
            tc.tile_pool(name="mtiles", bufs=2) as mtiles,
            tc.tile_pool(name="work", bufs=2) as work,
            tc.tile_pool(name="small", bufs=4) as small,
            tc.tile_pool(name="bigps", bufs=2, space="PSUM") as bigps,
            tc.tile_pool(name="mps", bufs=2, space="PSUM") as mps,
            tc.tile_pool(name="tpps", bufs=2, space="PSUM") as tpps,
        ):
            # ---- persistent tiles (DMA order: x16+A(0) first so M(0)
            # can start; fp8 pieces and C(0) land during it) ----
            x16 = persist.tile([P, ED, S], f16)
            nc.sync.dma_start(x16[:], x16_t)
            ones16 = persist.tile([1, 512], f16)
            nc.vector.memset(ones16[:], 1.0)
            ident = persist.tile([P, P], f16)
            make_identity(nc, ident)
            acc = persist.tile([P, SD, D], f32)
            recips = persist.tile([P, SD], f32)
            pT = persist.tile([P, SD, S], f16)
            nsb = persist.tile([P, SD, D], f16)

            def load_head(h):
                a18 = astream.tile([P, ED, D], f16, tag="a18")
                nc.sync.dma_start(a18[:], a18_t[h])
                w18 = astream.tile([1, D], f16, tag="w18")
                nc.sync.dma_start(w18[:], w18_d[h])
                if dr:
                    al8 = astream.tile([P, ED, D], f8, tag="al8")
                    nc.sync.dma_start(al8[:], al8_t[h])
                    a8 = astream.tile([P, ED, D], f8, tag="a8")
                    nc.sync.dma_start(a8[:], a8_t[h])
                    return (a18, w18, al8, a8)
                al16 = astream.tile([P, ED, D], f16, tag="al16")
                nc.sync.dma_start(al16[:], al16_t[h])
                return (a18, w18, al16)

            def load_c(h):
                c16 = astream.tile([P, ED, D], f16, tag="c16")
                nc.sync.dma_start(c16[:], c16_t[h])
                return c16

            def build_m(atiles):
                """M psum (scale 2^18) -> M16 (f16, 2^10), Ml8, M8 (f8)."""
                m16 = mtiles.tile([P, ED, S], f16, tag="m16")
                if dr:
                    ml8 = mtiles.tile([P, ED, S], f8, tag="ml8")
                    m8 = mtiles.tile([P, ED, S], f8, tag="m8")
                else:
                    ml16 = mtiles.tile([P, ED, S], f16, tag="ml16")
                for et in range(ED):
                    e_sl = slice(et * P, (et + 1) * P)
                    for sc in range(2):
                        s_sl = slice(sc * 512, (sc + 1) * 512)
                        ps = mps.tile([P, 512], f32, tag="m")
                        if dr:
                            a18, w18, al8, a8 = atiles
                            for dt_ in range(ED):
                                nc.tensor.matmul(
                                    ps[:], a18[:, dt_, e_sl], x16[:, dt_, s_sl],
                                    start=(dt_ == 0), stop=False)
                            # + 1 (x) w^T: column shift of scores, folded in
                            # at scale 2^18 so the hi/lo split carries it
                            nc.tensor.matmul(
                                ps[:], w18[:, e_sl], ones16[:],
                                start=False, stop=False)
                            for a in range(ED // 2):
                                d2 = slice(2 * a, 2 * a + 2)
                                nc.tensor.matmul(
                                    ps[:], al8[:, d2, e_sl], x8[:, d2, s_sl],
                                    start=False, stop=False, perf_mode=DR)
                            for a in range(ED // 2):
                                d2 = slice(2 * a, 2 * a + 2)
                                nc.tensor.matmul(
                                    ps[:], a8[:, d2, e_sl], xl12[:, d2, s_sl],
                                    start=False, stop=(a == ED // 2 - 1),
                                    perf_mode=DR)
                        else:
                            a18, w18, al16 = atiles
                            for dt_ in range(ED):
                                nc.tensor.matmul(
                                    ps[:], a18[:, dt_, e_sl], x16[:, dt_, s_sl],
                                    start=(dt_ == 0), stop=False)
                                nc.tensor.matmul(
                                    ps[:], al16[:, dt_, e_sl], x16[:, dt_, s_sl],
                                    start=False, stop=False)
                                nc.tensor.matmul(
                                    ps[:], a18[:, dt_, e_sl], xl16[:, dt_, s_sl],
                                    start=False, stop=False)
                            nc.tensor.matmul(
                                ps[:], w18[:, e_sl], ones16[:],
                                start=False, stop=True)
                        nc.scalar.activation(
                            m16[:, et, s_sl], ps[:], AF.Copy, scale=2.0 ** -8)
                        if dr:
                            nc.vector.scalar_tensor_tensor(
                                ml8[:, et, s_sl], ps[:], 2.0 ** -8,
                                m16[:, et, s_sl], ALU.mult, ALU.subtract)
                            nc.scalar.activation(
                                m8[:, et, s_sl], ps[:], AF.Copy, scale=2.0 ** -20)
                        else:
                            nc.vector.scalar_tensor_tensor(
                                ml16[:, et, s_sl], ps[:], 2.0 ** -8,
                                m16[:, et, s_sl], ALU.mult, ALU.subtract)
                if dr:
                    return (m16, ml8, m8)
                return (m16, ml16)

            def build_n(c16):
                """N = x C (fp16), layout [t-part, tt, d]."""
                n = nsb
                for tt in range(SD):
                    t_sl = slice(tt * P, (tt + 1) * P)
                    ps = bigps.tile([P, D], f32, tag="big")
                    for (d0, d1) in ((0, 512), (512, D)):
                        for et in range(ED):
                            nc.tensor.matmul(
                                ps[:, d0:d1], x16[:, et, t_sl],
                                c16[:, et, d0:d1],
                                start=(et == 0), stop=(et == ED - 1))
                    nc.scalar.activation(n[:, tt, :], ps[:], AF.Copy)
                return n

            def scores_tile(st, mt):
                """scores psum for s-tile st -> P (unnorm, f16) + recip."""
                s_sl = slice(st * P, (st + 1) * P)
                sc_ps = bigps.tile([P, S], f32, tag="big")
                for tch in range(2):
                    t_sl = slice(tch * 512, (tch + 1) * 512)
                    if dr:
                        m16, ml8, m8 = mt
                        for et in range(ED):
                            nc.tensor.matmul(
                                sc_ps[:, t_sl], m16[:, et, s_sl],
                                x16[:, et, t_sl], start=(et == 0), stop=False)
                        for a in range(ED // 2):
                            e2 = slice(2 * a, 2 * a + 2)
                            nc.tensor.matmul(
                                sc_ps[:, t_sl], ml8[:, e2, s_sl],
                                x8[:, e2, t_sl],
                                start=False, stop=False, perf_mode=DR)
                        for a in range(ED // 2):
                            e2 = slice(2 * a, 2 * a + 2)
                            nc.tensor.matmul(
                                sc_ps[:, t_sl], m8[:, e2, s_sl],
                                xl12[:, e2, t_sl],
                                start=False, stop=(a == ED // 2 - 1),
                                perf_mode=DR)
                    else:
                        m16, ml16 = mt
                        for et in range(ED):
                            nc.tensor.matmul(
                                sc_ps[:, t_sl], m16[:, et, s_sl],
                                x16[:, et, t_sl], start=(et == 0), stop=False)
                            nc.tensor.matmul(
                                sc_ps[:, t_sl], ml16[:, et, s_sl],
                                x16[:, et, t_sl], start=False, stop=False)
                            nc.tensor.matmul(
                                sc_ps[:, t_sl], m16[:, et, s_sl],
                                xl16[:, et, t_sl], start=False,
                                stop=(et == ED - 1))
                negmax = small.tile([P, 1], f32, tag="negmax")
                nc.vector.tensor_reduce(
                    negmax[:], sc_ps[:], axis=mybir.AxisListType.X,
                    op=mybir.AluOpType.max, negate=True)
                bias8 = small.tile([P, 1], f32, tag="bias8")
                nc.vector.tensor_scalar_mul(bias8[:], negmax[:], EXP_SCALE)
                ptile = work.tile([P, S], f16, tag="p")
                sumexp = small.tile([P, 1], f32, tag="sumexp")
                nc.scalar.activation(
                    ptile[:], sc_ps[:], AF.Exp,
                    bias=bias8[:], scale=EXP_SCALE, accum_out=sumexp[:])
                nc.vector.reciprocal(recips[:, st:st + 1], sumexp[:])
                return ptile

            def transpose_p(st, ptile):
                # P^T on the PE (a DMA-engine transpose degrades to 256B
                # packets SBUF->SBUF and backlogs the queue)
                s_sl = slice(st * P, (st + 1) * P)
                tp_ps = tpps.tile([P, SD, P], f16, tag="tp")
                for tt in range(SD):
                    t_sl = slice(tt * P, (tt + 1) * P)
                    nc.tensor.transpose(
                        tp_ps[:, tt, :], ptile[:, t_sl], ident[:])
                nc.vector.tensor_copy(pT[:, :, s_sl], tp_ps[:])

            def out_tile(st, n):
                s_sl = slice(st * P, (st + 1) * P)
                ps = bigps.tile([P, D], f32, tag="big")
                for (d0, d1) in ((0, 512), (512, D)):
                    for tt in range(SD):
                        nc.tensor.matmul(
                            ps[:, d0:d1], pT[:, tt, s_sl], n[:, tt, d0:d1],
                            start=(tt == 0), stop=(tt == SD - 1))
                nc.vector.scalar_tensor_tensor(
                    acc[:, st, :], ps[:], recips[:, st:st + 1],
                    acc[:, st, :], ALU.mult, ALU.add)

            # ---- prologue ----
            nc.vector.memset(acc[:], 0.0)
            atiles = load_head(0)
            if dr:
                x8 = persist.tile([P, ED, S], f8)
                nc.sync.dma_start(x8[:], x8_t)
                xl12 = persist.tile([P, ED, S], f8)
                nc.sync.dma_start(xl12[:], xl12_t)
            else:
                xl16 = persist.tile([P, ED, S], f16)
                nc.sync.dma_start(xl16[:], xl16_t)
            c16 = load_c(0)
            mt = build_m(atiles)
            n = build_n(c16)

            # ---- head loop (software-pipelined) ----
            for h in range(H):
                if h + 1 < H:
                    atiles_n = load_head(h + 1)
                    c16_n = load_c(h + 1)
                ptiles = {}
                for st in range(SD):
                    ptiles[st] = scores_tile(st, mt)
                    if st >= 1:
                        transpose_p(st - 1, ptiles[st - 1])
                        del ptiles[st - 1]
                if h + 1 < H:
                    mt_next = build_m(atiles_n)
                transpose_p(SD - 1, ptiles[SD - 1])
                for st in range(SD):
                    out_tile(st, n)
                    if h == H - 1:
                        nc.sync.dma_start(out_t[:, st, :], acc[:, st, :])
                if h + 1 < H:
                    mt = mt_next
                    n = build_n(c16_n)

    nc.compile()
    return nc


def _get_nc():
    if "nc" not in _CACHE:
        _CACHE["nc"] = _build_nc()
    return _CACHE["nc"]


def _prepare(x, Wq, bq, Wk, bk, Wv, bv, Wp, bp):
    x = np.asarray(x, dtype=np.float32)
    Wq = np.asarray(Wq, dtype=np.float32)
    Wk = np.asarray(Wk, dtype=np.float32)
    Wv = np.asarray(Wv, dtype=np.float32)
    Wp = np.asarray(Wp, dtype=np.float32).reshape(H, D, D)
    bq = np.asarray(bq, dtype=np.float32)
    bv = np.asarray(bv, dtype=np.float32)
    bp = np.asarray(bp, dtype=np.float32)

    # folded weights
    A = np.matmul(Wq, np.transpose(Wk, (0, 2, 1)))          # [H,D,D] x A x^T
    C = np.matmul(Wv, Wp)                                   # [H,D,D]
    w = np.einsum('hde,he->hd', Wk, bq)                     # [H,D] col shift
    bp_eff = (bp.astype(np.float64)
              + np.einsum('hd,hde->e', bv.astype(np.float64),
                          Wp.astype(np.float64))).astype(np.float32)

    A18f = A * np.float32(2.0 ** 18)
    A18 = np.clip(A18f, -65504, 65504).astype(np.float16)
    Alr = A18f - A18.astype(np.float32)
    C16 = C.astype(np.float16)

    shared = {"A18": A18, "C16": C16}
    if SCHEME == "dr":
        shared["Al8"] = np.clip(Alr, -240, 240).astype(E4)
        shared["A8"] = np.clip(A * np.float32(2.0 ** 6), -240, 240).astype(E4)
    else:
        shared["Al16"] = Alr.astype(np.float16)

    # column shift w at the M-stage 2^18 scale frame
    shared["W18"] = np.clip(w * np.float32(2.0 ** 18),
                            -65504, 65504).astype(np.float16)[:, None, :]

    in_maps = []
    for b in range(B):
        xT = np.ascontiguousarray(x[b].T)
        x16 = xT.astype(np.float16)
        xl = xT - x16.astype(np.float32)
        m = {"x16T": x16, **shared}
        if SCHEME == "dr":
            m["x8T"] = np.clip(xT, -240, 240).astype(E4)
            m["xl12T"] = np.clip(xl * np.float32(2.0 ** 12), -240, 240).astype(E4)
        else:
            m["xl16T"] = xl.astype(np.float16)
        in_maps.append(m)
    return in_maps, bp_eff


def kernel(x, Wq, bq, Wk, bk, Wv, bv, Wp, bp):
    from concourse.bass_utils import run_bass_kernel_spmd

    in_maps, bp_eff = _prepare(x, Wq, bq, Wk, bk, Wv, bv, Wp, bp)
    nc = _get_nc()
    res = run_bass_kernel_spmd(nc, in_maps, list(range(B)))
    out = np.stack([res.results[b]["out"] for b in range(B)], axis=0)
    out = out + bp_eff[None, None, :]
    return out.astype(np.float32)


# revision 47
# speedup vs baseline: 1.2674x; 1.0431x over previous
"""Trainium2 Bass kernel for nn_MultiHeadAttention_65352222376626.

Reference (B=8, S=1024, D=768, H=12):
    q = einsum('bsd,hde->bhse', x, Wq) + bq     (full-width per-head proj)
    k, v likewise
    scores = einsum('bhse,bhte->bhst', q, k) * sqrt(64)
    attn = softmax(scores, -1)
    o = einsum('bhst,bhte->bhse', attn, v)
    out = concat_heads(o) @ Wp + bp

Algebraic folding (softmax is invariant to per-row shifts; rows of attn
sum to 1):
    scores ~ x A_h x^T + 1 (x w_h)^T          A_h = Wq_h Wk_h^T,  w_h = Wk_h bq_h
    out    = sum_h attn_h (x C_h) + bp_eff    C_h = Wv_h Wp_h
    bp_eff = bp + sum_h bv_h Wp_h             (bq row-term and bk drop entirely)
This removes the separate q/k/v and output projections: per head only
  M = x A_h   [S,D]   and   scores = M x^T   [S,S]
  N = x C_h   [S,D]   and   out += attn N    [S,D]

Precision: the softmax is near-argmax (logit std ~222), so scores need
~2^-16 relative accuracy.  Each of the two scores-path matmuls runs as
3 passes: fp16(hi)*fp16(hi) at 1.0 cyc/row plus TWO fp8e4m3 DoubleRow
correction passes at 0.5 cyc/row (lo*hi and hi*lo), with power-of-2
scale frames chosen so every fp8 operand sits in e4m3's normal range:
  A frame 2^18:  A18=f16(A*2^18), Al8=e4(A*2^18-A18), A8=e4(A*2^6)
  x frame 2^0 :  x16=f16(x), x8=e4(x), xl12=e4((x-x16)*2^12)
  M psum at 2^18 -> M16=f16(psum*2^-8) [2^10], Ml8=e4(psum*2^-8-M16),
                    M8=e4(psum*2^-20) [2^-2]
  scores psum at 2^10; exp(psum*(8/1024) - 8/1024*rowmax)
Verified on hardware (probe_dr.py): HW == host simulation exactly;
end-to-end absmax rel err ~1.5e-3.  The column shift g = x w_h is
host-computed (G, scale 2^10) and broadcast-added into the scores psum
via a K=1 float32r ones-matmul.  attn/N/out path is plain fp16 (error
there is linear, not argmax-amplified).

Sharding: pure batch-parallel, B == n_cores == 8, one batch element per
core, folded weights replicated.  No collectives.

Schedule: per head, PE does scores(st=0..7) with softmax lagging on
DVE/ACT and P-transposes lagging one tile; M(h+1) fills the softmax
drain; then out(h) and N(h+1).  PSUM: big pool 2x2 banks (scores/N/out),
tp 2x1, M 2x1 -> 8 banks.
"""

import numpy as np
import ml_dtypes

B, S, D, H = 8, 1024, 768, 12
P = 128
SD = S // P   # 8 s-tiles
ED = D // P   # 6 feature tiles
EXP_SCALE = 8.0 / 8192.0   # sqrt(64) / (scores psum scale 2^13)

SCHEME = "dr"   # "dr" = fp16+fp8 DoubleRow corrections; "f16" = fp16 3-pass

_CACHE = {}

E4 = ml_dtypes.float8_e4m3


def _build_nc(scheme=SCHEME):
    import concourse.tile as tile
    from concourse import bacc, mybir
    from concourse.masks import make_identity

    f32 = mybir.dt.float32
    f16 = mybir.dt.float16
    f8 = mybir.dt.float8e4
    AF = mybir.ActivationFunctionType
    ALU = mybir.AluOpType
    DR = mybir.MatmulPerfMode.DoubleRow
    dr = scheme == "dr"

    nc = bacc.Bacc()

    # ---- DRAM I/O (per core) ----
    x16_d = nc.dram_tensor("x16T", [D, S], f16, kind="ExternalInput")
    if dr:
        x8_d = nc.dram_tensor("x8T", [D, S], f8, kind="ExternalInput")
        xl12_d = nc.dram_tensor("xl12T", [D, S], f8, kind="ExternalInput")
    else:
        xl16_d = nc.dram_tensor("xl16T", [D, S], f16, kind="ExternalInput")
    a18_d = nc.dram_tensor("A18", [H, D, D], f16, kind="ExternalInput")
    if dr:
        al8_d = nc.dram_tensor("Al8", [H, D, D], f8, kind="ExternalInput")
        a8_d = nc.dram_tensor("A8", [H, D, D], f8, kind="ExternalInput")
    else:
        al16_d = nc.dram_tensor("Al16", [H, D, D], f16, kind="ExternalInput")
    c16_d = nc.dram_tensor("C16", [H, D, D], f16, kind="ExternalInput")
    w13_d = nc.dram_tensor("W13", [H, P, ED], f32, kind="ExternalInput")
    w2_d = nc.dram_tensor("W2", [H, P, ED], f32, kind="ExternalInput")
    out_d = nc.dram_tensor("out", [S, D], f32, kind="ExternalOutput")

    # partition-tiled DRAM views
    tv = lambda d: d.rearrange("(o p) s -> p o s", p=P)          # [128,ED,S]
    wv = lambda d: d.rearrange("h (o p) e -> h p o e", p=P)      # [H,128,ED,D]
    x16_t = tv(x16_d)
    if dr:
        x8_t, xl12_t = tv(x8_d), tv(xl12_d)
    else:
        xl16_t = tv(xl16_d)
    a18_t = wv(a18_d)
    if dr:
        al8_t, a8_t = wv(al8_d), wv(a8_d)
    else:
        al16_t = wv(al16_d)
    c16_t = wv(c16_d)
    out_t = out_d.rearrange("(o p) d -> p o d", p=P)             # [128,SD,D]

    with tile.TileContext(nc) as tc:
        with (
            tc.tile_pool(name="persist", bufs=1) as persist,
            tc.tile_pool(name="astream", bufs=2) as astream,
            tc.tile_pool(name="mtiles", bufs=2) as mtiles,
            tc.tile_pool(name="work", bufs=2) as work,
            tc.tile_pool(name="small", bufs=4) as small,
            tc.tile_pool(name="bigps", bufs=2, space="PSUM") as bigps,
            tc.tile_pool(name="mps", bufs=2, space="PSUM") as mps,
            tc.tile_pool(name="tpps", bufs=2, space="PSUM") as tpps,
        ):
            # ---- persistent tiles (DMA order: x16+A(0) first so M(0)
            # can start; fp8 pieces and C(0) land during it) ----
            x16 = persist.tile([P, ED, S], f16)
            nc.sync.dma_start(x16[:], x16_t)
            ident = persist.tile([P, P], f16)
            make_identity(nc, ident)
            acc = persist.tile([P, SD, D], f32)
            recips = persist.tile([P, SD], f32)
            pT = persist.tile([P, SD, S], f16)
            nsb = persist.tile([P, SD, D], f16)

            def load_head(h):
                a18 = astream.tile([P, ED, D], f16, tag="a18")
                nc.sync.dma_start(a18[:], a18_t[h])
                w13 = astream.tile([P, ED], f32, tag="w13")
                nc.sync.dma_start(w13[:], w13_d[h])
                w2 = astream.tile([P, ED], f32, tag="w2")
                nc.sync.dma_start(w2[:], w2_d[h])
                if dr:
                    al8 = astream.tile([P, ED, D], f8, tag="al8")
                    nc.sync.dma_start(al8[:], al8_t[h])
                    a8 = astream.tile([P, ED, D], f8, tag="a8")
                    nc.sync.dma_start(a8[:], a8_t[h])
                    return (a18, w13, w2, al8, a8)
                al16 = astream.tile([P, ED, D], f16, tag="al16")
                nc.sync.dma_start(al16[:], al16_t[h])
                return (a18, w13, w2, al16)

            def load_c(h):
                c16 = astream.tile([P, ED, D], f16, tag="c16")
                nc.sync.dma_start(c16[:], c16_t[h])
                return c16

            def build_m(atiles):
                """M psum (scale 2^13, w folded via act biases) ->
                M16 (f16, 2^13), Ml8 (f8, 2^13), M8 (f8, 2^1)."""
                m16 = mtiles.tile([P, ED, S], f16, tag="m16")
                if dr:
                    ml8 = mtiles.tile([P, ED, S], f8, tag="ml8")
                    m8 = mtiles.tile([P, ED, S], f8, tag="m8")
                else:
                    ml16 = mtiles.tile([P, ED, S], f16, tag="ml16")
                for et in range(ED):
                    e_sl = slice(et * P, (et + 1) * P)
                    for sc in range(2):
                        s_sl = slice(sc * 512, (sc + 1) * 512)
                        ps = mps.tile([P, 512], f32, tag="m")
                        if dr:
                            a18, w13, w2, al8, a8 = atiles
                            for dt_ in range(ED):
                                nc.tensor.matmul(
                                    ps[:], a18[:, dt_, e_sl], x16[:, dt_, s_sl],
                                    start=(dt_ == 0), stop=False)
                            for a in range(ED // 2):
                                d2 = slice(2 * a, 2 * a + 2)
                                nc.tensor.matmul(
                                    ps[:], al8[:, d2, e_sl], x8[:, d2, s_sl],
                                    start=False, stop=False, perf_mode=DR)
                            for a in range(ED // 2):
                                d2 = slice(2 * a, 2 * a + 2)
                                nc.tensor.matmul(
                                    ps[:], a8[:, d2, e_sl], xl12[:, d2, s_sl],
                                    start=False, stop=(a == ED // 2 - 1),
                                    perf_mode=DR)
                        else:
                            a18, w13, w2, al16 = atiles
                            for dt_ in range(ED):
                                nc.tensor.matmul(
                                    ps[:], a18[:, dt_, e_sl], x16[:, dt_, s_sl],
                                    start=(dt_ == 0), stop=False)
                                nc.tensor.matmul(
                                    ps[:], al16[:, dt_, e_sl], x16[:, dt_, s_sl],
                                    start=False, stop=False)
                                nc.tensor.matmul(
                                    ps[:], a18[:, dt_, e_sl], xl16[:, dt_, s_sl],
                                    start=False, stop=(dt_ == ED - 1))
                        # w rides in on the psum->SBUF copies (per-partition
                        # bias = w[e]); Ml8 gets it via the AP-scalar add
                        wb13 = w13[:, et:et + 1]
                        nc.scalar.activation(
                            m16[:, et, s_sl], ps[:], AF.Identity, bias=wb13)
                        if dr:
                            nc.vector.scalar_tensor_tensor(
                                ml8[:, et, s_sl], ps[:], wb13,
                                m16[:, et, s_sl], ALU.add, ALU.subtract)
                            nc.scalar.activation(
                                m8[:, et, s_sl], ps[:], AF.Identity,
                                scale=2.0 ** -12, bias=w2[:, et:et + 1])
                        else:
                            nc.vector.scalar_tensor_tensor(
                                ml16[:, et, s_sl], ps[:], wb13,
                                m16[:, et, s_sl], ALU.add, ALU.subtract)
                if dr:
                    return (m16, ml8, m8)
                return (m16, ml16)

            def build_n(c16):
                """N = x C (fp16), layout [t-part, tt, d]."""
                n = nsb
                for tt in range(SD):
                    t_sl = slice(tt * P, (tt + 1) * P)
                    ps = bigps.tile([P, D], f32, tag="big")
                    for (d0, d1) in ((0, 512), (512, D)):
                        for et in range(ED):
                            nc.tensor.matmul(
                                ps[:, d0:d1], x16[:, et, t_sl],
                                c16[:, et, d0:d1],
                                start=(et == 0), stop=(et == ED - 1))
                    nc.scalar.activation(n[:, tt, :], ps[:], AF.Copy)
                return n

            def scores_tile(st, mt):
                """scores psum for s-tile st -> P (unnorm, f16) + recip."""
                s_sl = slice(st * P, (st + 1) * P)
                sc_ps = bigps.tile([P, S], f32, tag="big")
                for tch in range(2):
                    t_sl = slice(tch * 512, (tch + 1) * 512)
                    if dr:
                        m16, ml8, m8 = mt
                        for et in range(ED):
                            nc.tensor.matmul(
                                sc_ps[:, t_sl], m16[:, et, s_sl],
                                x16[:, et, t_sl], start=(et == 0), stop=False)
                        for a in range(ED // 2):
                            e2 = slice(2 * a, 2 * a + 2)
                            nc.tensor.matmul(
                                sc_ps[:, t_sl], ml8[:, e2, s_sl],
                                x8[:, e2, t_sl],
                                start=False, stop=False, perf_mode=DR)
                        for a in range(ED // 2):
                            e2 = slice(2 * a, 2 * a + 2)
                            nc.tensor.matmul(
                                sc_ps[:, t_sl], m8[:, e2, s_sl],
                                xl12[:, e2, t_sl],
                                start=False, stop=(a == ED // 2 - 1),
                                perf_mode=DR)
                    else:
                        m16, ml16 = mt
                        for et in range(ED):
                            nc.tensor.matmul(
                                sc_ps[:, t_sl], m16[:, et, s_sl],
                                x16[:, et, t_sl], start=(et == 0), stop=False)
                            nc.tensor.matmul(
                                sc_ps[:, t_sl], ml16[:, et, s_sl],
                                x16[:, et, t_sl], start=False, stop=False)
                            nc.tensor.matmul(
                                sc_ps[:, t_sl], m16[:, et, s_sl],
                                xl16[:, et, t_sl], start=False,
                                stop=(et == ED - 1))
                negmax = small.tile([P, 1], f32, tag="negmax")
                nc.vector.tensor_reduce(
                    negmax[:], sc_ps[:], axis=mybir.AxisListType.X,
                    op=mybir.AluOpType.max, negate=True)
                bias8 = small.tile([P, 1], f32, tag="bias8")
                nc.vector.tensor_scalar_mul(bias8[:], negmax[:], EXP_SCALE)
                ptile = work.tile([P, S], f16, tag="p")
                sumexp = small.tile([P, 1], f32, tag="sumexp")
                nc.scalar.activation(
                    ptile[:], sc_ps[:], AF.Exp,
                    bias=bias8[:], scale=EXP_SCALE, accum_out=sumexp[:])
                nc.vector.reciprocal(recips[:, st:st + 1], sumexp[:])
                return ptile

            def transpose_p(st, ptile):
                # P^T on the PE (a DMA-engine transpose degrades to 256B
                # packets SBUF->SBUF and backlogs the queue)
                s_sl = slice(st * P, (st + 1) * P)
                tp_ps = tpps.tile([P, SD, P], f16, tag="tp")
                for tt in range(SD):
                    t_sl = slice(tt * P, (tt + 1) * P)
                    nc.tensor.transpose(
                        tp_ps[:, tt, :], ptile[:, t_sl], ident[:])
                nc.vector.tensor_copy(pT[:, :, s_sl], tp_ps[:])

            def out_tile(st, n):
                s_sl = slice(st * P, (st + 1) * P)
                ps = bigps.tile([P, D], f32, tag="big")
                for (d0, d1) in ((0, 512), (512, D)):
                    for tt in range(SD):
                        nc.tensor.matmul(
                            ps[:, d0:d1], pT[:, tt, s_sl], n[:, tt, d0:d1],
                            start=(tt == 0), stop=(tt == SD - 1))
                nc.vector.scalar_tensor_tensor(
                    acc[:, st, :], ps[:], recips[:, st:st + 1],
                    acc[:, st, :], ALU.mult, ALU.add)

            # ---- prologue ----
            nc.vector.memset(acc[:], 0.0)
            atiles = load_head(0)
            if dr:
                x8 = persist.tile([P, ED, S], f8)
                nc.sync.dma_start(x8[:], x8_t)
                xl12 = persist.tile([P, ED, S], f8)
                nc.sync.dma_start(xl12[:], xl12_t)
            else:
                xl16 = persist.tile([P, ED, S], f16)
                nc.sync.dma_start(xl16[:], xl16_t)
            c16 = load_c(0)
            mt = build_m(atiles)
            n = build_n(c16)

            # ---- head loop (software-pipelined) ----
            for h in range(H):
                if h + 1 < H:
                    atiles_n = load_head(h + 1)
                    c16_n = load_c(h + 1)
                ptiles = {}
                for st in range(SD):
                    ptiles[st] = scores_tile(st, mt)
                    if st >= 1:
                        transpose_p(st - 1, ptiles[st - 1])
                        del ptiles[st - 1]
                if h + 1 < H:
                    mt_next = build_m(atiles_n)
                transpose_p(SD - 1, ptiles[SD - 1])
                for st in range(SD):
                    out_tile(st, n)
                    if h == H - 1:
                        nc.sync.dma_start(out_t[:, st, :], acc[:, st, :])
                if h + 1 < H:
                    mt = mt_next
                    n = build_n(c16_n)

    nc.compile()
    return nc


def _get_nc():
    if "nc" not in _CACHE:
        _CACHE["nc"] = _build_nc()
    return _CACHE["nc"]


def _prepare(x, Wq, bq, Wk, bk, Wv, bv, Wp, bp):
    x = np.asarray(x, dtype=np.float32)
    Wq = np.asarray(Wq, dtype=np.float32)
    Wk = np.asarray(Wk, dtype=np.float32)
    Wv = np.asarray(Wv, dtype=np.float32)
    Wp = np.asarray(Wp, dtype=np.float32).reshape(H, D, D)
    bq = np.asarray(bq, dtype=np.float32)
    bv = np.asarray(bv, dtype=np.float32)
    bp = np.asarray(bp, dtype=np.float32)

    # folded weights
    A = np.matmul(Wq, np.transpose(Wk, (0, 2, 1)))          # [H,D,D] x A x^T
    C = np.matmul(Wv, Wp)                                   # [H,D,D]
    w = np.einsum('hde,he->hd', Wk, bq)                     # [H,D] col shift
    bp_eff = (bp.astype(np.float64)
              + np.einsum('hd,hde->e', bv.astype(np.float64),
                          Wp.astype(np.float64))).astype(np.float32)

    A13f = A * np.float32(2.0 ** 13)
    A18 = np.clip(A13f, -65504, 65504).astype(np.float16)
    Alr = A13f - A18.astype(np.float32)
    C16 = C.astype(np.float16)

    shared = {"A18": A18, "C16": C16}
    if SCHEME == "dr":
        shared["Al8"] = np.clip(Alr, -240, 240).astype(E4)
        shared["A8"] = np.clip(A * np.float32(2.0 ** 1), -240, 240).astype(E4)
    else:
        shared["Al16"] = Alr.astype(np.float16)

    # column shift w, delivered as per-partition act biases in the
    # M-stage frames (2^13 for M16/Ml8, 2^1 for M8)
    wcol = np.ascontiguousarray(
        w.reshape(H, ED, P).transpose(0, 2, 1)).astype(np.float32)
    shared["W13"] = wcol * np.float32(2.0 ** 13)
    shared["W2"] = wcol * np.float32(2.0 ** 1)

    in_maps = []
    for b in range(B):
        xT = np.ascontiguousarray(x[b].T)
        x16 = xT.astype(np.float16)
        xl = xT - x16.astype(np.float32)
        m = {"x16T": x16, **shared}
        if SCHEME == "dr":
            m["x8T"] = np.clip(xT, -240, 240).astype(E4)
            m["xl12T"] = np.clip(xl * np.float32(2.0 ** 12), -240, 240).astype(E4)
        else:
            m["xl16T"] = xl.astype(np.float16)
        in_maps.append(m)
    return in_maps, bp_eff


def kernel(x, Wq, bq, Wk, bk, Wv, bv, Wp, bp):
    from concourse.bass_utils import run_bass_kernel_spmd

    in_maps, bp_eff = _prepare(x, Wq, bq, Wk, bk, Wv, bv, Wp, bp)
    nc = _get_nc()
    res = run_bass_kernel_spmd(nc, in_maps, list(range(B)))
    out = np.stack([res.results[b]["out"] for b in range(B)], axis=0)
    out = out + bp_eff[None, None, :]
    return out.astype(np.float32)


# revision 51
# speedup vs baseline: 1.2700x; 1.0021x over previous
"""Trainium2 Bass kernel for nn_MultiHeadAttention_65352222376626.

Reference (B=8, S=1024, D=768, H=12):
    q = einsum('bsd,hde->bhse', x, Wq) + bq     (full-width per-head proj)
    k, v likewise
    scores = einsum('bhse,bhte->bhst', q, k) * sqrt(64)
    attn = softmax(scores, -1)
    o = einsum('bhst,bhte->bhse', attn, v)
    out = concat_heads(o) @ Wp + bp

Algebraic folding (softmax is invariant to per-row shifts; rows of attn
sum to 1):
    scores ~ x A_h x^T + 1 (x w_h)^T          A_h = Wq_h Wk_h^T,  w_h = Wk_h bq_h
    out    = sum_h attn_h (x C_h) + bp_eff    C_h = Wv_h Wp_h
    bp_eff = bp + sum_h bv_h Wp_h             (bq row-term and bk drop entirely)
This removes the separate q/k/v and output projections: per head only
  M = x A_h   [S,D]   and   scores = M x^T   [S,S]
  N = x C_h   [S,D]   and   out += attn N    [S,D]

Precision: the softmax is near-argmax (logit std ~222), so scores need
~2^-16 relative accuracy.  Each of the two scores-path matmuls runs as
3 passes: fp16(hi)*fp16(hi) at 1.0 cyc/row plus TWO fp8e4m3 DoubleRow
correction passes (K=256/instruction, ~1.9x fp16 MAC rate), with
power-of-2 scale frames chosen so every fp8 operand sits in e4m3's
normal range:
  A frame 2^13:  A18=f16(A*2^13), Al8=e4(A*2^13-A18), A8=e4(A*2^1)
  x frame 2^0 :  x16=f16(x), x8=e4(x), xl12=e4((x-x16)*2^12)
  M psum at 2^13 -> M16=f16(psum+w13) [2^13],
                    Ml8=e4((psum+w13)-M16) [2^13],
                    M8=e4(psum*2^-12+w2) [2^1]
  scores psum at 2^13; exp(psum*(8/8192) - 8/8192*rowmax)
The column shift w_h = Wk_h bq_h rides in as per-partition activation /
scalar_tensor_tensor biases on the M psum drains (zero extra matmuls).
Verified on hardware (probe_dr.py): HW == host simulation exactly;
end-to-end absmax rel err ~1.4e-3.  attn/N/out path is plain fp16
(error there is linear, not argmax-amplified).

Sharding: pure batch-parallel, B == n_cores == 8, one batch element per
core, folded weights replicated.  No collectives.

Schedule: per head, PE does scores(st=0..7) with softmax lagging on
DVE/ACT and P-transposes lagging one tile; M(h+1) fills the softmax
drain; then out(h) and N(h+1).  PSUM: big pool 2x2 banks (scores/N/out),
tp 2x1, M 2x1 -> 8 banks.
"""

import numpy as np
import ml_dtypes

B, S, D, H = 8, 1024, 768, 12
P = 128
SD = S // P   # 8 s-tiles
ED = D // P   # 6 feature tiles
EXP_SCALE = 8.0 / 8192.0   # sqrt(64) / (scores psum scale 2^13)

SCHEME = "dr"   # "dr" = fp16+fp8 DoubleRow corrections; "f16" = fp16 3-pass

_CACHE = {}

E4 = ml_dtypes.float8_e4m3


def _build_nc(scheme=SCHEME):
    import concourse.tile as tile
    from concourse import bacc, mybir
    from concourse.masks import make_identity

    f32 = mybir.dt.float32
    f16 = mybir.dt.float16
    f8 = mybir.dt.float8e4
    AF = mybir.ActivationFunctionType
    ALU = mybir.AluOpType
    DR = mybir.MatmulPerfMode.DoubleRow
    dr = scheme == "dr"

    nc = bacc.Bacc()

    # ---- DRAM I/O (per core) ----
    x16_d = nc.dram_tensor("x16T", [D, S], f16, kind="ExternalInput")
    if dr:
        x8_d = nc.dram_tensor("x8T", [D, S], f8, kind="ExternalInput")
        xl12_d = nc.dram_tensor("xl12T", [D, S], f8, kind="ExternalInput")
    else:
        xl16_d = nc.dram_tensor("xl16T", [D, S], f16, kind="ExternalInput")
    a18_d = nc.dram_tensor("A18", [H, D, D], f16, kind="ExternalInput")
    if dr:
        al8_d = nc.dram_tensor("Al8", [H, D, D], f8, kind="ExternalInput")
        a8_d = nc.dram_tensor("A8", [H, D, D], f8, kind="ExternalInput")
    else:
        al16_d = nc.dram_tensor("Al16", [H, D, D], f16, kind="ExternalInput")
    c16_d = nc.dram_tensor("C16", [H, D, D], f16, kind="ExternalInput")
    w13_d = nc.dram_tensor("W13", [H, P, ED], f32, kind="ExternalInput")
    w2_d = nc.dram_tensor("W2", [H, P, ED], f32, kind="ExternalInput")
    out_d = nc.dram_tensor("out", [S, D], f32, kind="ExternalOutput")

    # partition-tiled DRAM views
    tv = lambda d: d.rearrange("(o p) s -> p o s", p=P)          # [128,ED,S]
    wv = lambda d: d.rearrange("h (o p) e -> h p o e", p=P)      # [H,128,ED,D]
    x16_t = tv(x16_d)
    if dr:
        x8_t, xl12_t = tv(x8_d), tv(xl12_d)
    else:
        xl16_t = tv(xl16_d)
    a18_t = wv(a18_d)
    if dr:
        al8_t, a8_t = wv(al8_d), wv(a8_d)
    else:
        al16_t = wv(al16_d)
    c16_t = wv(c16_d)
    out_t = out_d.rearrange("(o p) d -> p o d", p=P)             # [128,SD,D]

    with tile.TileContext(nc) as tc:
        with (
            tc.tile_pool(name="persist", bufs=1) as persist,
            tc.tile_pool(name="astream", bufs=2) as astream,
            tc.tile_pool(name="mtiles", bufs=2) as mtiles,
            tc.tile_pool(name="work", bufs=2) as work,
            tc.tile_pool(name="small", bufs=4) as small,
            tc.tile_pool(name="bigps", bufs=2, space="PSUM") as bigps,
            tc.tile_pool(name="mps", bufs=2, space="PSUM") as mps,
            tc.tile_pool(name="tpps", bufs=2, space="PSUM") as tpps,
        ):
            # ---- persistent tiles (DMA order: x16+A(0) first so M(0)
            # can start; fp8 pieces and C(0) land during it) ----
            x16 = persist.tile([P, ED, S], f16)
            nc.sync.dma_start(x16[:, :, 0:512], x16_t[:, :, 0:512])
            nc.sync.dma_start(x16[:, :, 512:S], x16_t[:, :, 512:S])
            ident = persist.tile([P, P], f16)
            make_identity(nc, ident)
            acc = persist.tile([P, SD, D], f32)
            recips = persist.tile([P, SD], f32)
            pT = persist.tile([P, SD, S], f16)
            nsb = persist.tile([P, SD, D], f16)

            def load_head(h):
                a18 = astream.tile([P, ED, D], f16, tag="a18")
                nc.sync.dma_start(a18[:], a18_t[h])
                w13 = astream.tile([P, ED], f32, tag="w13")
                nc.sync.dma_start(w13[:], w13_d[h])
                w2 = astream.tile([P, ED], f32, tag="w2")
                nc.sync.dma_start(w2[:], w2_d[h])
                if dr:
                    al8 = astream.tile([P, ED, D], f8, tag="al8")
                    nc.sync.dma_start(al8[:], al8_t[h])
                    a8 = astream.tile([P, ED, D], f8, tag="a8")
                    nc.sync.dma_start(a8[:], a8_t[h])
                    return (a18, w13, w2, al8, a8)
                al16 = astream.tile([P, ED, D], f16, tag="al16")
                nc.sync.dma_start(al16[:], al16_t[h])
                return (a18, w13, w2, al16)

            def load_c(h):
                c16 = astream.tile([P, ED, D], f16, tag="c16")
                nc.sync.dma_start(c16[:], c16_t[h])
                return c16

            def build_m(atiles):
                """M psum (scale 2^13, w folded via act biases) ->
                M16 (f16, 2^13), Ml8 (f8, 2^13), M8 (f8, 2^1)."""
                m16 = mtiles.tile([P, ED, S], f16, tag="m16")
                if dr:
                    ml8 = mtiles.tile([P, ED, S], f8, tag="ml8")
                    m8 = mtiles.tile([P, ED, S], f8, tag="m8")
                else:
                    ml16 = mtiles.tile([P, ED, S], f16, tag="ml16")
                for et in range(ED):
                    e_sl = slice(et * P, (et + 1) * P)
                    for sc in range(2):
                        s_sl = slice(sc * 512, (sc + 1) * 512)
                        ps = mps.tile([P, 512], f32, tag="m")
                        if dr:
                            a18, w13, w2, al8, a8 = atiles
                            for dt_ in range(ED):
                                nc.tensor.matmul(
                                    ps[:], a18[:, dt_, e_sl], x16[:, dt_, s_sl],
                                    start=(dt_ == 0), stop=False)
                            for a in range(ED // 2):
                                d2 = slice(2 * a, 2 * a + 2)
                                nc.tensor.matmul(
                                    ps[:], al8[:, d2, e_sl], x8[:, d2, s_sl],
                                    start=False, stop=False, perf_mode=DR)
                            for a in range(ED // 2):
                                d2 = slice(2 * a, 2 * a + 2)
                                nc.tensor.matmul(
                                    ps[:], a8[:, d2, e_sl], xl12[:, d2, s_sl],
                                    start=False, stop=(a == ED // 2 - 1),
                                    perf_mode=DR)
                        else:
                            a18, w13, w2, al16 = atiles
                            for dt_ in range(ED):
                                nc.tensor.matmul(
                                    ps[:], a18[:, dt_, e_sl], x16[:, dt_, s_sl],
                                    start=(dt_ == 0), stop=False)
                                nc.tensor.matmul(
                                    ps[:], al16[:, dt_, e_sl], x16[:, dt_, s_sl],
                                    start=False, stop=False)
                                nc.tensor.matmul(
                                    ps[:], a18[:, dt_, e_sl], xl16[:, dt_, s_sl],
                                    start=False, stop=(dt_ == ED - 1))
                        # w rides in on the psum->SBUF copies (per-partition
                        # bias = w[e]); Ml8 gets it via the AP-scalar add
                        wb13 = w13[:, et:et + 1]
                        nc.scalar.activation(
                            m16[:, et, s_sl], ps[:], AF.Identity, bias=wb13)
                        if dr:
                            nc.vector.scalar_tensor_tensor(
                                ml8[:, et, s_sl], ps[:], wb13,
                                m16[:, et, s_sl], ALU.add, ALU.subtract)
                            nc.scalar.activation(
                                m8[:, et, s_sl], ps[:], AF.Identity,
                                scale=2.0 ** -12, bias=w2[:, et:et + 1])
                        else:
                            nc.vector.scalar_tensor_tensor(
                                ml16[:, et, s_sl], ps[:], wb13,
                                m16[:, et, s_sl], ALU.add, ALU.subtract)
                if dr:
                    return (m16, ml8, m8)
                return (m16, ml16)

            def build_n(c16):
                """N = x C (fp16), layout [t-part, tt, d]."""
                n = nsb
                for tt in range(SD):
                    t_sl = slice(tt * P, (tt + 1) * P)
                    ps = bigps.tile([P, D], f32, tag="big")
                    for (d0, d1) in ((0, 512), (512, D)):
                        for et in range(ED):
                            nc.tensor.matmul(
                                ps[:, d0:d1], x16[:, et, t_sl],
                                c16[:, et, d0:d1],
                                start=(et == 0), stop=(et == ED - 1))
                    nc.scalar.activation(n[:, tt, :], ps[:], AF.Copy)
                return n

            def scores_tile(st, mt):
                """scores psum for s-tile st -> P (unnorm, f16) + recip."""
                s_sl = slice(st * P, (st + 1) * P)
                sc_ps = bigps.tile([P, S], f32, tag="big")
                for tch in range(2):
                    t_sl = slice(tch * 512, (tch + 1) * 512)
                    if dr:
                        m16, ml8, m8 = mt
                        for et in range(ED):
                            nc.tensor.matmul(
                                sc_ps[:, t_sl], m16[:, et, s_sl],
                                x16[:, et, t_sl], start=(et == 0), stop=False)
                        for a in range(ED // 2):
                            e2 = slice(2 * a, 2 * a + 2)
                            nc.tensor.matmul(
                                sc_ps[:, t_sl], ml8[:, e2, s_sl],
                                x8[:, e2, t_sl],
                                start=False, stop=False, perf_mode=DR)
                        for a in range(ED // 2):
                            e2 = slice(2 * a, 2 * a + 2)
                            nc.tensor.matmul(
                                sc_ps[:, t_sl], m8[:, e2, s_sl],
                                xl12[:, e2, t_sl],
                                start=False, stop=(a == ED // 2 - 1),
                                perf_mode=DR)
                    else:
                        m16, ml16 = mt
                        for et in range(ED):
                            nc.tensor.matmul(
                                sc_ps[:, t_sl], m16[:, et, s_sl],
                                x16[:, et, t_sl], start=(et == 0), stop=False)
                            nc.tensor.matmul(
                                sc_ps[:, t_sl], ml16[:, et, s_sl],
                                x16[:, et, t_sl], start=False, stop=False)
                            nc.tensor.matmul(
                                sc_ps[:, t_sl], m16[:, et, s_sl],
                                xl16[:, et, t_sl], start=False,
                                stop=(et == ED - 1))
                negmax = small.tile([P, 1], f32, tag="negmax")
                nc.vector.tensor_reduce(
                    negmax[:], sc_ps[:], axis=mybir.AxisListType.X,
                    op=mybir.AluOpType.max, negate=True)
                bias8 = small.tile([P, 1], f32, tag="bias8")
                nc.vector.tensor_scalar_mul(bias8[:], negmax[:], EXP_SCALE)
                ptile = work.tile([P, S], f16, tag="p")
                sumexp = small.tile([P, 1], f32, tag="sumexp")
                nc.scalar.activation(
                    ptile[:], sc_ps[:], AF.Exp,
                    bias=bias8[:], scale=EXP_SCALE, accum_out=sumexp[:])
                nc.vector.reciprocal(recips[:, st:st + 1], sumexp[:])
                return ptile

            def transpose_p(st, ptile):
                # P^T on the PE (a DMA-engine transpose degrades to 256B
                # packets SBUF->SBUF and backlogs the queue)
                s_sl = slice(st * P, (st + 1) * P)
                tp_ps = tpps.tile([P, SD, P], f16, tag="tp")
                for tt in range(SD):
                    t_sl = slice(tt * P, (tt + 1) * P)
                    nc.tensor.transpose(
                        tp_ps[:, tt, :], ptile[:, t_sl], ident[:])
                nc.vector.tensor_copy(pT[:, :, s_sl], tp_ps[:])

            def out_tile(st, n):
                s_sl = slice(st * P, (st + 1) * P)
                ps = bigps.tile([P, D], f32, tag="big")
                for (d0, d1) in ((0, 512), (512, D)):
                    for tt in range(SD):
                        nc.tensor.matmul(
                            ps[:, d0:d1], pT[:, tt, s_sl], n[:, tt, d0:d1],
                            start=(tt == 0), stop=(tt == SD - 1))
                nc.vector.scalar_tensor_tensor(
                    acc[:, st, :], ps[:], recips[:, st:st + 1],
                    acc[:, st, :], ALU.mult, ALU.add)

            # ---- prologue ----
            nc.vector.memset(acc[:], 0.0)
            atiles = load_head(0)
            if dr:
                x8 = persist.tile([P, ED, S], f8)
                nc.sync.dma_start(x8[:], x8_t)
                xl12 = persist.tile([P, ED, S], f8)
                nc.sync.dma_start(xl12[:], xl12_t)
            else:
                xl16 = persist.tile([P, ED, S], f16)
                nc.sync.dma_start(xl16[:], xl16_t)
            c16 = load_c(0)
            mt = build_m(atiles)
            n = build_n(c16)

            # ---- head loop (software-pipelined) ----
            for h in range(H):
                if h + 1 < H:
                    atiles_n = load_head(h + 1)
                    c16_n = load_c(h + 1)
                # out(st) only needs pT's st column-block, so while the
                # last s-tile's softmax drains, the PE is kept busy with
                # M(h+1) (steady state) or out(0..6) (last head).
                ptiles = {}
                for st in range(SD):
                    ptiles[st] = scores_tile(st, mt)
                    if st >= 1:
                        transpose_p(st - 1, ptiles[st - 1])
                        del ptiles[st - 1]
                if h + 1 < H:
                    mt_next = build_m(atiles_n)
                else:
                    for st in range(SD - 1):
                        out_tile(st, n)
                        nc.sync.dma_start(out_t[:, st, :], acc[:, st, :])
                transpose_p(SD - 1, ptiles[SD - 1])
                if h + 1 < H:
                    for st in range(SD):
                        out_tile(st, n)
                    mt = mt_next
                    n = build_n(c16_n)
                else:
                    out_tile(SD - 1, n)
                    nc.sync.dma_start(
                        out_t[:, SD - 1, :], acc[:, SD - 1, :])

    nc.compile()
    return nc


def _get_nc():
    if "nc" not in _CACHE:
        _CACHE["nc"] = _build_nc()
    return _CACHE["nc"]


def _prepare(x, Wq, bq, Wk, bk, Wv, bv, Wp, bp):
    x = np.asarray(x, dtype=np.float32)
    Wq = np.asarray(Wq, dtype=np.float32)
    Wk = np.asarray(Wk, dtype=np.float32)
    Wv = np.asarray(Wv, dtype=np.float32)
    Wp = np.asarray(Wp, dtype=np.float32).reshape(H, D, D)
    bq = np.asarray(bq, dtype=np.float32)
    bv = np.asarray(bv, dtype=np.float32)
    bp = np.asarray(bp, dtype=np.float32)

    # folded weights
    A = np.matmul(Wq, np.transpose(Wk, (0, 2, 1)))          # [H,D,D] x A x^T
    C = np.matmul(Wv, Wp)                                   # [H,D,D]
    w = np.einsum('hde,he->hd', Wk, bq)                     # [H,D] col shift
    bp_eff = (bp.astype(np.float64)
              + np.einsum('hd,hde->e', bv.astype(np.float64),
                          Wp.astype(np.float64))).astype(np.float32)

    A13f = A * np.float32(2.0 ** 13)
    A18 = np.clip(A13f, -65504, 65504).astype(np.float16)
    Alr = A13f - A18.astype(np.float32)
    C16 = C.astype(np.float16)

    shared = {"A18": A18, "C16": C16}
    if SCHEME == "dr":
        shared["Al8"] = np.clip(Alr, -240, 240).astype(E4)
        shared["A8"] = np.clip(A * np.float32(2.0 ** 1), -240, 240).astype(E4)
    else:
        shared["Al16"] = Alr.astype(np.float16)

    # column shift w, delivered as per-partition act biases in the
    # M-stage frames (2^13 for M16/Ml8, 2^1 for M8)
    wcol = np.ascontiguousarray(
        w.reshape(H, ED, P).transpose(0, 2, 1)).astype(np.float32)
    shared["W13"] = wcol * np.float32(2.0 ** 13)
    shared["W2"] = wcol * np.float32(2.0 ** 1)

    in_maps = []
    for b in range(B):
        xT = np.ascontiguousarray(x[b].T)
        x16 = xT.astype(np.float16)
        xl = xT - x16.astype(np.float32)
        m = {"x16T": x16, **shared}
        if SCHEME == "dr":
            m["x8T"] = np.clip(xT, -240, 240).astype(E4)
            m["xl12T"] = np.clip(xl * np.float32(2.0 ** 12), -240, 240).astype(E4)
        else:
            m["xl16T"] = xl.astype(np.float16)
        in_maps.append(m)
    return in_maps, bp_eff


def kernel(x, Wq, bq, Wk, bk, Wv, bv, Wp, bp):
    from concourse.bass_utils import run_bass_kernel_spmd

    in_maps, bp_eff = _prepare(x, Wq, bq, Wk, bk, Wv, bv, Wp, bp)
    nc = _get_nc()
    res = run_bass_kernel_spmd(nc, in_maps, list(range(B)))
    out = np.stack([res.results[b]["out"] for b in range(B)], axis=0)
    out = out + bp_eff[None, None, :]
    return out.astype(np.float32)
